# revision 5
# baseline (speedup 1.0000x reference)
"""Edge-MLP GNN message passing kernel for Trainium2 (8 NeuronCores).

Computes, for each edge e = (u, v):
    out[e] = sigmoid(relu(|x[u] - x[v]| @ W1 + b1) @ W2 + b2)

Strategy (data parallel over edges, x + weights replicated):
  - Per core (80000 edges): dma_gather(transpose=True) fetches both
    endpoints' feature rows as [128 feat, C edges] columns in SBUF
    (the DMA XBar does the transpose for free).
  - Gathers round-robin over 4 SWDGE queues: queue q's descriptors are
    generated by Q7 core pair (2q, 2q+1), so 4 queues = 4x parallel
    descriptor generation (the single-queue baseline bottleneck).
  - single_packet=True makes each gather's descriptor stream one packet
    per SDMA engine, so engines don't interleave different gathers'
    streams mid-XBar-tile (interleaving corrupts transposed gathers).
  - Optionally (SRC_SBUF) x lives in SBUF in a token/rank stripe layout
    and the gather never touches HBM.
  - DVE: d = g0 - g1, |d| on ACT; PE: h = W1.T @ d with two 512-edge
    groups packed into PSUM partition halves; DVE: relu(h + b1) -> fp16;
    PE: per 128-edge group matmul(h[64,128], W2[64,1]) -> PSUM column;
    ACT: sigmoid(+ b2) -> out_sb; one DMA to DRAM at the end.
  - Host reassembles: out[p, c] = edge c*128 + p per core.
"""

import os
import sys

for _p in ("/opt/trn_rl_repo", "/root/.axon_site/_ro/trn_rl_repo"):
    if os.path.isdir(_p) and _p not in sys.path:
        sys.path.insert(0, _p)

import numpy as np

import concourse.bacc as bacc
import concourse.mybir as mybir
from concourse.mybir import AluOpType
from concourse.tile import TileContext
from concourse.bass_utils import run_bass_kernel_spmd

N_NODES = 10000
N_EDGES = 640000
D_FEAT = 128
HID = 64
N_CORES = 8
E_CORE = N_EDGES // N_CORES  # 80000 edges per core

CHUNK = 8192  # edges per gather chunk (must be % 128)
N_QUEUES = 4
SINGLE_PACKET = True
SRC_SBUF = True
SCRATCH = 32768

RANKS = (N_NODES + 127) // 128  # 79
NPAD = RANKS * 128  # 10112

f16 = mybir.dt.float16
f32 = mybir.dt.float32
i16 = mybir.dt.int16

_NC_CACHE = None


def _chunk_list(e_core, chunk):
    chunks = []
    rem = e_core
    while rem > 0:
        c = min(chunk, rem)
        chunks.append(c)
        rem -= c
    return chunks


def _build_nc(e_core=E_CORE, chunk=CHUNK):
    """Build + finalize the (SPMD, per-core identical) Bass kernel."""
    _chunks = _chunk_list(e_core, chunk)
    n_out_cols = e_core // 128

    nc = bacc.Bacc(
        "TRN2",
        target_bir_lowering=False,
        num_swdge_queues=N_QUEUES,
        dynamic_dma_scratch_size=SCRATCH,
    )

    if SRC_SBUF:
        # Stripe layout: node i -> partition i % 128, bytes
        # [(i//128)*256, +256) of the free dim (sbuf_tokens_per_rank=128,
        # sbuf_free_dim_per_rank=256B).
        xsb_d = nc.dram_tensor("xsb", [128, RANKS * D_FEAT], f16,
                               kind="ExternalInput")
    else:
        x16 = nc.dram_tensor("x16", [N_NODES, D_FEAT], f16, kind="ExternalInput")
    idx0_d = nc.dram_tensor("idx0", [128, e_core // 16], i16, kind="ExternalInput")
    idx1_d = nc.dram_tensor("idx1", [128, e_core // 16], i16, kind="ExternalInput")
    w1_d = nc.dram_tensor("w1", [D_FEAT, HID], f16, kind="ExternalInput")
    w2_d = nc.dram_tensor("w2", [128, 1], f16, kind="ExternalInput")  # W2 stacked 2x
    b1_d = nc.dram_tensor("b1", [128, 1], f32, kind="ExternalInput")  # b1 stacked 2x
    b2_d = nc.dram_tensor("b2", [128, 1], f32, kind="ExternalInput")  # b2 bcast
    out_d = nc.dram_tensor("out", [128, n_out_cols], f32, kind="ExternalOutput")

    with TileContext(nc) as tc:
        with (
            tc.tile_pool(name="const", bufs=1) as cpool,
            tc.tile_pool(name="gather", bufs=6) as gpool,
            tc.tile_pool(name="diff", bufs=2) as dpool,
            tc.tile_pool(name="hid", bufs=4) as hpool,
            tc.tile_pool(name="outp", bufs=1) as opool,
            tc.tile_pool(name="ps1", bufs=4, space="PSUM") as ppool,
            tc.tile_pool(name="ps2", bufs=2, space="PSUM") as p2pool,
        ):
            idx0 = cpool.tile([128, e_core // 16], i16, tag="idx0")
            idx1 = cpool.tile([128, e_core // 16], i16, tag="idx1")
            w1 = cpool.tile([D_FEAT, HID], f16, tag="w1")
            w2 = cpool.tile([128, 1], f16, tag="w2")
            b1 = cpool.tile([128, 1], f32, tag="b1")
            b2 = cpool.tile([128, 1], f32, tag="b2")
            out_sb = opool.tile([128, n_out_cols], f32, tag="osb")
            if SRC_SBUF:
                xsb = cpool.tile([128, RANKS * D_FEAT], f16, tag="xsb")
                nc.sync.dma_start(xsb[:], xsb_d[:])

            nc.sync.dma_start(idx0[:], idx0_d[:])
            nc.sync.dma_start(idx1[:], idx1_d[:])
            nc.sync.dma_start(w1[:], w1_d[:])
            nc.sync.dma_start(w2[:], w2_d[:])
            nc.sync.dma_start(b1[:], b1_d[:])
            nc.sync.dma_start(b2[:], b2_d[:])

            def gather(dst, idx_t, c0, C, q):
                kw = {}
                if SRC_SBUF:
                    src = xsb[:]
                    kw = dict(
                        sbuf_tokens_per_rank=128,
                        sbuf_free_dim_per_rank=D_FEAT * 2,
                    )
                else:
                    src = x16[:]
                nc.gpsimd.dma_gather(
                    dst[:].rearrange("p (a c) -> p a c", a=1),
                    src,
                    idx_t[:, c0 : c0 + C // 16],
                    C,
                    C,
                    elem_size=D_FEAT,
                    transpose=True,
                    single_packet=SINGLE_PACKET,
                    queue_num=q,
                    **kw,
                )

            e0 = 0  # first edge of chunk
            col0 = 0  # first out_sb column of chunk
            n_gathers = 0
            for C in _chunks:
                g0 = gpool.tile([128, C], f16, tag="g0")
                g1 = gpool.tile([128, C], f16, tag="g1")
                c0 = e0 // 16
                gather(g0, idx0, c0, C, n_gathers % N_QUEUES)
                n_gathers += 1
                gather(g1, idx1, c0, C, n_gathers % N_QUEUES)
                n_gathers += 1
                d = dpool.tile([128, C], f16, tag="d")
                nc.vector.tensor_tensor(d[:], g0[:], g1[:], AluOpType.subtract)
                # |d| on the (otherwise idle) scalar engine
                nc.scalar.activation(
                    d[:], d[:], mybir.ActivationFunctionType.Abs,
                )

                ncols = C // 128
                p2 = p2pool.tile([128, ncols], f32, tag="p2")
                colc = 0
                for g in range(0, C, 1024):
                    nA = min(512, C - g)
                    nB = min(512, C - g - nA)
                    pm = ppool.tile([128, 512], f32, tag="pm")
                    nc.tensor.matmul(
                        pm[0:HID, 0:nA], w1[:], d[:, g : g + nA],
                        start=True, stop=True,
                    )
                    if nB:
                        nc.tensor.matmul(
                            pm[HID:128, 0:nB], w1[:], d[:, g + nA : g + nA + nB],
                            start=True, stop=True,
                        )
                    h = hpool.tile([128, 512], f16, tag="h")
                    if nB == nA:
                        nc.vector.tensor_scalar(
                            h[:, 0:nA], pm[:, 0:nA], b1[:], 0.0,
                            AluOpType.add, AluOpType.max,
                        )
                    else:
                        nc.vector.tensor_scalar(
                            h[0:HID, 0:nA], pm[0:HID, 0:nA], b1[0:HID, :], 0.0,
                            AluOpType.add, AluOpType.max,
                        )
                        if nB:
                            nc.vector.tensor_scalar(
                                h[HID:128, 0:nB], pm[HID:128, 0:nB], b1[HID:128, :],
                                0.0, AluOpType.add, AluOpType.max,
                            )
                    for j in range(nA // 128):
                        nc.tensor.matmul(
                            p2[:, colc : colc + 1],
                            h[0:HID, j * 128 : (j + 1) * 128],
                            w2[0:HID, :],
                            start=True, stop=True,
                        )
                        colc += 1
                    for j in range(nB // 128):
                        nc.tensor.matmul(
                            p2[:, colc : colc + 1],
                            h[HID:128, j * 128 : (j + 1) * 128],
                            w2[HID:128, :],
                            start=True, stop=True,
                        )
                        colc += 1
                nc.scalar.activation(
                    out_sb[:, col0 : col0 + ncols], p2[:, 0:ncols],
                    mybir.ActivationFunctionType.Sigmoid,
                    bias=b2[:], scale=1.0,
                )
                e0 += C
                col0 += ncols

            nc.sync.dma_start(out_d[:], out_sb[:])

    nc.finalize()
    return nc


def _get_nc():
    global _NC_CACHE
    if _NC_CACHE is None:
        _NC_CACHE = _build_nc()
    return _NC_CACHE


def _interleave_idx(a):
    """[e_core] int array -> [128, e_core//16] int16 SWDGE index layout.

    dma_gather consumes index i from [i % 16, i // 16]; rows are wrapped in
    16 partitions and replicated 8x for the 8 Q7 cores.
    """
    e_core = a.shape[0]
    m = a.reshape(e_core // 16, 16).T.astype(np.int16)  # [16, E/16]
    return np.tile(m, (8, 1))  # [128, E/16]


def prep_in_maps(x, indices, W1, b1, W2, b2):
    x16 = np.ascontiguousarray(np.asarray(x, dtype=np.float32)).astype(np.float16)
    idx = np.asarray(indices)
    w1 = np.asarray(W1, dtype=np.float32).astype(np.float16)
    w2c = np.asarray(W2, dtype=np.float32).astype(np.float16).reshape(HID, 1)
    w2s = np.concatenate([w2c, w2c], axis=0)  # [128, 1]
    b1c = np.asarray(b1, dtype=np.float32).reshape(HID, 1)
    b1s = np.concatenate([b1c, b1c], axis=0)  # [128, 1]
    b2s = np.full((128, 1), np.asarray(b2, dtype=np.float32).reshape(-1)[0],
                  dtype=np.float32)

    if SRC_SBUF:
        xpad = np.zeros((NPAD, D_FEAT), np.float16)
        xpad[:N_NODES] = x16
        xsb = np.ascontiguousarray(
            xpad.reshape(RANKS, 128, D_FEAT).transpose(1, 0, 2).reshape(
                128, RANKS * D_FEAT)
        )

    in_maps = []
    for c in range(N_CORES):
        sl = slice(c * E_CORE, (c + 1) * E_CORE)
        m = {
            "idx0": _interleave_idx(idx[0, sl]),
            "idx1": _interleave_idx(idx[1, sl]),
            "w1": w1,
            "w2": w2s,
            "b1": b1s,
            "b2": b2s,
        }
        if SRC_SBUF:
            m["xsb"] = xsb
        else:
            m["x16"] = x16
        in_maps.append(m)
    return in_maps


def run_hw(x, indices, W1, b1, W2, b2, trace=False, **kw):
    """Run on the 8 NeuronCores; returns (out [N_EDGES] f32, BassKernelResults)."""
    nc = _get_nc()
    in_maps = prep_in_maps(x, indices, W1, b1, W2, b2)
    res = run_bass_kernel_spmd(
        nc, in_maps, core_ids=list(range(N_CORES)), trace=trace, **kw
    )
    outs = []
    for c in range(N_CORES):
        o = np.asarray(res.results[c]["out"])  # [128, 625]
        outs.append(o.T.reshape(-1))  # edge e = col*128 + p
    return np.concatenate(outs), res


def kernel(x, indices, W1, b1, W2, b2):
    out, _ = run_hw(x, indices, W1, b1, W2, b2, trace=False)
    return out.astype(np.float32)
